# revision 1
# baseline (speedup 1.0000x reference)
"""Trainium2 Bass kernel: single-head causal attention (B=4, T=4096, C=2048, H=128).

    q = x @ Wq; k = x @ Wk; v = x @ Wv        (per batch element)
    out = softmax(causal(q k^T * C**-0.5)) @ v

Sharding: two cores per batch element (8 cores, B=4). Within a batch the
4096 q rows are split between the pair by 128-row-block parity (core p
owns blocks p, p+2, ...), which balances the causal work. Each core
projects k/v only for its own row blocks; the halves are exchanged with a
pair-local AllGather. Attention runs in scores-transposed layout: for a
512-row q superblock, S^T[k,q] = kT_blk^T @ qT on the tensor engine,
P = exp(S^T * scale) on the scalar engine (exp(x) is safe un-maxed here:
|scaled scores| < ~2), causal masking is a multiplicative 0/1 mask on the
diagonal blocks only, and O[q,:] accumulates P_chunk^T @ [V | 1] in PSUM
— the appended ones-column yields the softmax denominators for free.
"""

import numpy as np
import ml_dtypes

B, T, C, H = 4, 4096, 2048, 128
NCORES = 8
TQ = T // 2              # per-core q rows
NCC = C // 128           # 16 contraction chunks
NSB = TQ // 512          # 4 q superblocks of 512 rows per core
SCALE = float(C) ** -0.5
BF16 = ml_dtypes.bfloat16

# mask slots for the diagonal region: (dd, c) with dd = k-block offset in
# the 8-block diagonal band, c = q chunk index; a multiply is needed only
# when dd >= 2c (otherwise the block is fully allowed for both cores).
SLOTS = [(dd, c) for dd in range(8) for c in range(4) if dd >= 2 * c]
SLOT_IDX = {s: i for i, s in enumerate(SLOTS)}
NSLOT = len(SLOTS)  # 20

_cached = {}


def _build_nc(debug=False):
    import concourse.bacc as bacc
    import concourse.mybir as mybir
    from concourse import tile

    f32 = mybir.dt.float32
    bf16 = mybir.dt.bfloat16
    AF = mybir.ActivationFunctionType

    nc = bacc.Bacc("TRN2", target_bir_lowering=False, debug=False,
                   num_devices=NCORES)

    xT = nc.declare_dram_parameter("xT", [C, TQ], bf16, isOutput=False)
    wq = nc.declare_dram_parameter("Wq", [C, H], bf16, isOutput=False)
    wk = nc.declare_dram_parameter("Wk", [C, H], bf16, isOutput=False)
    wv = nc.declare_dram_parameter("Wv", [C, H], bf16, isOutput=False)
    msk = nc.declare_dram_parameter("masks", [128, NSLOT * 128], bf16,
                                    isOutput=False)
    out = nc.declare_dram_parameter("out", [TQ, H], f32, isOutput=True)
    if debug:
        dbg_kt = nc.declare_dram_parameter("dbg_kt", [128, T], bf16,
                                           isOutput=True)
        dbg_v = nc.declare_dram_parameter("dbg_v", [128, 2 * 16 * 130], bf16,
                                          isOutput=True)
        dbg_qt = nc.declare_dram_parameter("dbg_qt", [128, TQ], bf16,
                                           isOutput=True)
        dbg_den = nc.declare_dram_parameter("dbg_den", [128, 16], f32,
                                            isOutput=True)
        dbg_p = nc.declare_dram_parameter("dbg_p", [128, 4096], bf16,
                                          isOutput=True)
        dbg_o = nc.declare_dram_parameter("dbg_o", [128, 516], f32,
                                          isOutput=True)

    kT_half = nc.dram_tensor("kT_half", [128, TQ], bf16)
    v_half = nc.dram_tensor("v_half", [TQ, H], bf16)
    kT_full = nc.dram_tensor("kT_full", [256, TQ], bf16)
    v_full = nc.dram_tensor("v_full", [T, H], bf16)
    groups = [[0, 1], [2, 3], [4, 5], [6, 7]]

    with tile.TileContext(nc) as tc:
        with tc.tile_pool(name="sb", bufs=1) as sb, \
             tc.tile_pool(name="sbs", bufs=4) as sbs, \
             tc.tile_pool(name="p_sb", bufs=3) as p_pool, \
             tc.tile_pool(name="o_sb", bufs=6) as o_pool:

            # ---- resident loads -------------------------------------
            xT_sb = sb.tile([128, NCC * TQ], bf16)
            xTv = xT_sb[:].rearrange("p (n t) -> p n t", t=TQ)
            nc.sync.dma_start(xTv, xT.ap().rearrange("(n p) t -> p n t", p=128))

            w_sb = {}
            for name, h in (("wq", wq), ("wk", wk), ("wv", wv)):
                t = sb.tile([128, NCC * H], bf16, tag=name)
                nc.sync.dma_start(
                    t[:].rearrange("p (n h) -> p n h", h=H),
                    h.ap().rearrange("(n p) h -> p n h", p=128))
                w_sb[name] = t

            mask_sb = sb.tile([128, NSLOT * 128], bf16)
            nc.sync.dma_start(mask_sb[:], msk.ap())

            def wchunk(name, cc):
                return w_sb[name][:, cc * H:(cc + 1) * H]

            # ---- k^T projection for our half, then AllGather --------
            kTh_sb = sb.tile([128, TQ], bf16)
            with tc.tile_pool(name="ps_k", bufs=1, space="PSUM") as pskp:
                psk = [pskp.tile([128, 512], f32, tag=f"psk{g}", name=f"psk{g}")
                       for g in range(4)]
                for cc in range(NCC):
                    for g in range(4):
                        nc.tensor.matmul(
                            psk[g][:], wchunk("wk", cc),
                            xTv[:, cc, 512 * g:512 * (g + 1)],
                            start=(cc == 0), stop=(cc == NCC - 1))
                for g in range(4):
                    nc.scalar.copy(kTh_sb[:, 512 * g:512 * (g + 1)], psk[g][:])
            nc.sync.dma_start(kT_half.ap(), kTh_sb[:])
            nc.gpsimd.collective_compute(
                "AllGather", mybir.AluOpType.bypass, replica_groups=groups,
                ins=[kT_half.ap().opt()], outs=[kT_full.ap().opt()])

            # ---- v projection for our half, then AllGather ----------
            vh_sb = sb.tile([128, 16 * H], bf16)
            vhv = vh_sb[:].rearrange("p (n h) -> p n h", h=H)
            with tc.tile_pool(name="ps_v", bufs=4, space="PSUM") as psvp:
                for i in range(16):
                    psv = psvp.tile([128, H], f32)
                    for cc in range(NCC):
                        nc.tensor.matmul(
                            psv[:], xTv[:, cc, 128 * i:128 * (i + 1)],
                            wchunk("wv", cc),
                            start=(cc == 0), stop=(cc == NCC - 1))
                    nc.vector.tensor_copy(vhv[:, i, :], psv[:])
            nc.sync.dma_start(
                v_half.ap().rearrange("(n p) h -> p n h", p=128), vhv)
            nc.gpsimd.collective_compute(
                "AllGather", mybir.AluOpType.bypass, replica_groups=groups,
                ins=[v_half.ap().opt()], outs=[v_full.ap().opt()])

            # ---- q^T projection (overlaps the collectives) ----------
            qT_sb = sb.tile([128, TQ], bf16)
            with tc.tile_pool(name="ps_q", bufs=1, space="PSUM") as psqp:
                psq = [psqp.tile([128, 512], f32, tag=f"psq{g}", name=f"psq{g}")
                       for g in range(4)]
                for cc in range(NCC):
                    for g in range(4):
                        nc.tensor.matmul(
                            psq[g][:], wchunk("wq", cc),
                            xTv[:, cc, 512 * g:512 * (g + 1)],
                            start=(cc == 0), stop=(cc == NCC - 1))
                for g in range(4):
                    nc.scalar.copy(qT_sb[:, 512 * g:512 * (g + 1)], psq[g][:])

            # ---- load back the gathered full k^T / v ----------------
            kTf_sb = sb.tile([128, T], bf16)
            for half in range(2):
                nc.sync.dma_start(kTf_sb[:, TQ * half:TQ * (half + 1)],
                                  kT_full.ap()[128 * half:128 * (half + 1), :])
            v_all = sb.tile([128, 2 * 16 * 130], bf16)
            vav = v_all[:].rearrange("p (a n f) -> p a n f", a=2, f=130)
            for half in range(2):
                nc.sync.dma_start(
                    vav[:, half, :, 0:128],
                    v_full.ap()[TQ * half:TQ * half + TQ, :]
                    .rearrange("(n p) h -> p n h", p=128))
            nc.gpsimd.memset(vav[:, :, :, 128:129], 1.0)

            if debug:
                nc.sync.dma_start(dbg_kt.ap(), kTf_sb[:])
                nc.sync.dma_start(dbg_v.ap(), v_all[:])
                nc.sync.dma_start(dbg_qt.ap(), qT_sb[:])

            def kt_blk(j):
                base = TQ * (j % 2) + 128 * (j // 2)
                return kTf_sb[:, base:base + 128]

            def v_blk(j):
                return vav[:, j % 2, j // 2, 0:129]

            # ---- attention ------------------------------------------
            with tc.tile_pool(name="ps_s", bufs=2, space="PSUM") as pssp, \
                 tc.tile_pool(name="ps_o", bufs=2, space="PSUM") as psop:

                def o_chunk(tiles, c):
                    t = tiles[c // 2]
                    off = 129 * (c % 2)
                    return t[:, off:off + 129]

                pending = None  # (s, u, npair, P, o_tiles)

                def emit_av(p):
                    s, u, npair, P, o_tiles = p
                    for half in range(2):
                        j = 2 * u + half
                        for c in range(4):
                            nc.tensor.matmul(
                                o_chunk(o_tiles, c),
                                P[:, 512 * half + 128 * c:
                                   512 * half + 128 * (c + 1)],
                                v_blk(j),
                                # start clears the whole PSUM bank: only the
                                # first chunk written to each bank may set it
                                start=(u == 0 and half == 0 and c % 2 == 0),
                                stop=(u == npair - 1 and half == 1
                                      and c % 2 == 1),
                                skip_group_check=True)
                    if u == npair - 1:
                        # superblock finished: normalize + store
                        for c in range(4):
                            po = o_chunk(o_tiles, c)
                            rec = sbs.tile([128, 1], f32, tag="rec")
                            nc.vector.reciprocal(rec[:], po[:, 128:129])
                            if debug:
                                den = sbs.tile([128, 1], f32, tag="den",
                                               name=f"den{s}_{c}")
                                nc.vector.tensor_copy(den[:], po[:, 128:129])
                                nc.sync.dma_start(
                                    dbg_den.ap()[:, 4 * s + c:4 * s + c + 1],
                                    den[:])
                                if s == 0:
                                    oc = o_pool.tile([128, 129], f32,
                                                     tag="odbg",
                                                     name=f"odbg{c}")
                                    nc.vector.tensor_copy(oc[:], po[:])
                                    nc.sync.dma_start(
                                        dbg_o.ap()[:, 129 * c:129 * (c + 1)],
                                        oc[:])
                            osb = o_pool.tile([128, H], f32, tag="osb")
                            nc.vector.tensor_scalar_mul(
                                osb[:], po[:, 0:128], rec[:])
                            r0 = 128 * (4 * s + c)
                            nc.sync.dma_start(out.ap()[r0:r0 + 128, :], osb[:])

                for s in range(NSB):
                    o_tiles = (psop.tile([128, 258], f32, tag="oa", name=f"oa{s}"),
                               psop.tile([128, 258], f32, tag="ob", name=f"ob{s}"))
                    npair = 4 * s + 4
                    for u in range(npair):
                        pss = pssp.tile([128, 1024], f32)
                        for half in range(2):
                            j = 2 * u + half
                            nc.tensor.matmul(
                                pss[:, 512 * half:512 * (half + 1)],
                                kt_blk(j), qT_sb[:, 512 * s:512 * (s + 1)],
                                start=True, stop=True, skip_group_check=True)
                        P = p_pool.tile([128, 1024], bf16)
                        nc.scalar.activation(P[:], pss[:], AF.Exp, scale=SCALE)
                        if u >= 4 * s:
                            for half in range(2):
                                dd = 2 * (u - 4 * s) + half
                                for c in range(4):
                                    if dd >= 2 * c:
                                        si = SLOT_IDX[(dd, c)]
                                        pc = P[:, 512 * half + 128 * c:
                                               512 * half + 128 * (c + 1)]
                                        nc.vector.tensor_mul(
                                            pc, pc,
                                            mask_sb[:, 128 * si:128 * (si + 1)])
                        if debug and s == 0:
                            nc.sync.dma_start(
                                dbg_p.ap()[:, 1024 * u:1024 * (u + 1)], P[:])
                        if pending is not None:
                            emit_av(pending)
                        pending = (s, u, npair, P, o_tiles)
                if pending is not None:
                    emit_av(pending)

    nc.finalize()
    return nc


def _build_masks(p):
    kk = np.arange(128)[:, None]
    tt = np.arange(128)[None, :]
    tri = (kk <= tt).astype(np.float32)
    ones = np.ones((128, 128), np.float32)
    zero = np.zeros((128, 128), np.float32)
    M = np.zeros((128, NSLOT * 128), np.float32)
    for idx, (dd, c) in enumerate(SLOTS):
        if p == 0:
            m = tri if dd == 2 * c else zero
        else:
            m = ones if dd == 2 * c else (tri if dd == 2 * c + 1 else zero)
        M[:, idx * 128:(idx + 1) * 128] = m
    return np.ascontiguousarray(M.astype(BF16))


def _get_nc():
    if "nc" not in _cached:
        _cached["nc"] = _build_nc()
        _cached["masks"] = {p: _build_masks(p) for p in (0, 1)}
    return _cached["nc"]


def _prep_in_maps(x, Wq, Wk, Wv):
    _get_nc()
    w16 = {n: np.ascontiguousarray(np.asarray(w).astype(BF16))
           for n, w in (("Wq", Wq), ("Wk", Wk), ("Wv", Wv))}
    in_maps = []
    for c in range(NCORES):
        b, p = divmod(c, 2)
        xb = np.asarray(x[b]).reshape(T // 128, 128, C)[p::2].reshape(TQ, C)
        xTc = np.ascontiguousarray(xb.astype(BF16).T)
        in_maps.append({"xT": xTc, "masks": _cached["masks"][p], **w16})
    return in_maps


def _gather_out(results):
    out = np.empty((B, T, H), np.float32)
    for c in range(NCORES):
        b, p = divmod(c, 2)
        out[b].reshape(T // 128, 128, H)[p::2] = \
            results[c]["out"].reshape(TQ // 128, 128, H)
    return out


def kernel(x, Wq, Wk, Wv):
    from concourse.bass_utils import run_bass_kernel_spmd

    nc = _get_nc()
    in_maps = _prep_in_maps(x, Wq, Wk, Wv)
    res = run_bass_kernel_spmd(nc, in_maps, list(range(NCORES)))
    return _gather_out(res.results)

